# revision 1
# baseline (speedup 1.0000x reference)
"""Trainium2 Bass kernel for nn_ExpertGroup (moe_routing).

Contract: kernel(**inputs) takes FULL unsharded numpy inputs and returns the
FULL [2, 2048, 1024] fp32 output. Internally shards B*S=4096 tokens across
8 NeuronCores (512 tokens/core; cores 0-3 own batch 0, cores 4-7 batch 1),
replicates the small weights, and exchanges the per-batch adapter tensors
(adapt_in / adapt_out, [S,128] each) with two intra-group AllGathers.

All matmuls run in bf16 with fp32 PSUM accumulation. Activations are laid out
feature-major ([feature, token]) so every matmul contracts over partitions.

v2 schedule (vs v1 at 184us):
  - ug weights packed hc-major so the up/gate loop streams against DMA
    instead of stalling on the full 8.4MB load
  - expert path injected into the up/gate loop (its PSUM borrows the
    ps_out banks which are idle until final_down)
  - AO layernorm uses a single Rsqrt activation instead of
    sqrt + 1-partition DVE reciprocal (which cost 3.2us)
  - collective staging DMAs moved off the gpsimd queue (which blocks on
    the mesh entry barrier); warm-up payload is a host-provided zeros
    input so the first collective triggers immediately
  - AllGather outputs are Shared-address-space DRAM
"""

import sys

sys.path.insert(0, "/opt/trn_rl_repo")

import ml_dtypes
import numpy as np

import concourse.bass as bass
import concourse.mybir as mybir
import concourse.tile as tile
from concourse import bacc
from concourse.bass_utils import run_bass_kernel_spmd

BF16 = mybir.dt.bfloat16
F32 = mybir.dt.float32

B, S, D, E = 2, 2048, 1024, 8
H = 2 * D          # 2048
A = H // 16        # 128
N = B * S          # 4096
NCORES = 8
T = N // NCORES    # 512 tokens per core
GROUP = 4          # cores per batch
SC = T // 128      # 4 s-chunks per core
DC = D // 128      # 8 d-chunks (output features)
HC = H // 128      # 16 h-chunks
KD = D // 128      # 8 k-chunks over D
TC_FULL = S // 128  # 16 token-chunks per batch
EPS = 1e-5

_CACHE = {}


def _build():
    nc = bacc.Bacc(None, num_devices=NCORES)

    # ---- kernel I/O (per-core; weights pre-packed to SBUF layout on host) ----
    xT_d = nc.dram_tensor("xT", [128, KD, T], BF16, kind="ExternalInput")
    ug_d = nc.dram_tensor("ug_hc", [128, HC, KD * 256], BF16, kind="ExternalInput")
    pre_d = nc.dram_tensor("pre_wT", [128, KD, A], BF16, kind="ExternalInput")
    post_d = nc.dram_tensor("post_wT", [128, A, HC], BF16, kind="ExternalInput")
    adw_d = nc.dram_tensor("adapter_wT", [A, E * A], BF16, kind="ExternalInput")
    wfin_d = nc.dram_tensor("wfin", [128, DC, (HC + 2) * 128], BF16, kind="ExternalInput")
    ew_d = nc.dram_tensor("ew", [128, SC, E], F32, kind="ExternalInput")
    angb_d = nc.dram_tensor("angb", [2, A], F32, kind="ExternalInput")
    ancol_d = nc.dram_tensor("ancol", [128, 2], F32, kind="ExternalInput")
    ag_d = nc.dram_tensor("ag_row", [1, A * E], BF16, kind="ExternalInput")
    bmix_d = nc.dram_tensor("bias_mix", [128, SC, A], BF16, kind="ExternalInput")
    out_d = nc.dram_tensor("out", [D, T], F32, kind="ExternalOutput")

    # ---- collective bounce buffers (internal DRAM) ----
    ag1_in = nc.dram_tensor("ag1_in", [T, A], BF16)
    ag1_out = nc.dram_tensor("ag1_out", [S, A], BF16)
    ag2_in = nc.dram_tensor("ag2_in", [A, T], BF16)
    rrow_d = nc.dram_tensor("rrow", [1, T], BF16)
    nmrrow_d = nc.dram_tensor("nmrrow", [1, T], BF16)
    ag2_out = nc.dram_tensor("ag2_out", [GROUP * A, T], BF16)
    RG = [[0, 1, 2, 3], [4, 5, 6, 7]]

    with tile.TileContext(nc) as tc:
        with (
            tc.tile_pool(name="consts", bufs=1) as consts,
            tc.tile_pool(name="wpool", bufs=1) as wpool,
            tc.tile_pool(name="acts", bufs=1) as acts,
            tc.tile_pool(name="work", bufs=4) as work,
            tc.tile_pool(name="work2", bufs=2) as work2,
            tc.tile_pool(name="wtp", bufs=3) as wtp,
            tc.tile_pool(name="workbig", bufs=1) as workbig,
            tc.tile_pool(name="aoln", bufs=1) as aoln,
            tc.tile_pool(name="evac", bufs=2) as evac,
            tc.tile_pool(name="ps_big", bufs=3, space="PSUM") as ps_big,
            tc.tile_pool(name="ps_po", bufs=1, space="PSUM") as ps_po,
            tc.tile_pool(name="ps_acc", bufs=1, space="PSUM") as ps_acc,
            tc.tile_pool(name="ps_out", bufs=2, space="PSUM") as ps_out,
            tc.tile_pool(name="ps_sm", bufs=1, space="PSUM") as ps_sm,
        ):
            # No separate warm-up collective: a pending collective starves the
            # DMA rings until its mesh completes (measured: sync-queue DMAs
            # stall until collective end). AG1 itself absorbs the all-core
            # entry barrier -- it triggers at ~16us and is consumed at ~140us.

            # ---------- priority loads: the pre-matmul chain needs these ----
            pre_w = wpool.tile([128, KD, A], BF16)
            nc.sync.dma_start(out=pre_w, in_=pre_d[:])
            xT = wpool.tile([128, KD, T], BF16)
            for k in range(KD):
                nc.sync.dma_start(out=xT[:, k, :], in_=xT_d[:, k, :])
            # first ug chunks ahead of the consts so up/gate can start early
            ug_w = wpool.tile([128, HC, KD, 256], BF16, tag="bigw")
            for hc in range(3):
                nc.sync.dma_start(out=ug_w[:, hc], in_=ug_d[:, hc, :])

            # ---------- small consts + expert weights (needed mid-upgate) ----
            gB = consts.tile([128, A], F32)   # an_g broadcast across partitions
            bB = consts.tile([128, A], F32)   # an_b broadcast
            nc.sync.dma_start(
                out=gB,
                in_=bass.AP(tensor=angb_d, offset=0, ap=[[0, 128], [1, A]]),
            )
            nc.sync.dma_start(
                out=bB,
                in_=bass.AP(tensor=angb_d, offset=A, ap=[[0, 128], [1, A]]),
            )
            agB = consts.tile([128, E, A], BF16)  # adapter_g (e-major) bcast
            nc.sync.dma_start(
                out=agB,
                in_=bass.AP(tensor=ag_d, offset=0, ap=[[0, 128], [A, E], [1, A]]),
            )
            ancol = consts.tile([128, 2], F32)
            nc.sync.dma_start(out=ancol, in_=ancol_d[:])
            ew_sb = consts.tile([128, SC, E], F32)
            nc.sync.dma_start(out=ew_sb, in_=ew_d[:])
            bmix_sb = consts.tile([128, SC, A], BF16)
            nc.sync.dma_start(out=bmix_sb, in_=bmix_d[:])
            adw = wpool.tile([128, E * A], BF16)
            nc.sync.dma_start(out=adw, in_=adw_d[:])
            post_w = wpool.tile([128, A, HC], BF16)
            nc.sync.dma_start(out=post_w, in_=post_d[:])

            # ---------- streamed weights: ug per-hc (matches consume order) --
            # wfin shares ug_w's 64KB slot (tag "bigw"): its per-dc loads WAR-
            # wait on the ug reads they overlap, streaming in behind the
            # up/gate loop just in time for final_down.
            for hc in range(3, HC):
                nc.sync.dma_start(out=ug_w[:, hc], in_=ug_d[:, hc, :])
            wfin = wpool.tile([128, DC, HC + 2, 128], BF16, tag="bigw")
            for dc in range(DC):
                nc.sync.dma_start(out=wfin[:, dc], in_=wfin_d[:, dc, :])

            # ---------- memset constants (vector queue) ----------
            eps_t = consts.tile([128, 1], F32)
            nc.vector.memset(eps_t, EPS)
            ones_col = consts.tile([128, 1], BF16)
            nc.vector.memset(ones_col, 1.0)

            # persistent activations
            AI_tok = acts.tile([128, SC, A], BF16)    # adapt_in, token-major
            AIT = acts.tile([128, T], BF16)           # adapt_in, feature-major
            hid = acts.tile([128, HC, T], BF16)       # hidden, feature-major
            AOTfull = acts.tile([128, GROUP, T], BF16)   # gathered AO feat-major
            AOT = acts.tile([128, T], BF16)           # local AO, feature-major
            adaptT = acts.tile([128, T], BF16)        # adapt, feature-major
            mixedT = acts.tile([128, T], BF16)        # mixed, feature-major
            mix_tok = acts.tile([128, SC, A], BF16)   # mixed, token-major
            facc = acts.tile([128, DC, T], F32)       # down-part accumulator

            def layernorm_to(ps, dst):
                """LN over free dim (A=128) of psum tile [128, A]; write dst bf16."""
                st = work.tile([128, 6], F32, tag="lnst")
                nc.vector.bn_stats(out=st, in_=ps)
                mv = work.tile([128, 2], F32, tag="lnmv")
                nc.vector.bn_aggr(out=mv, in_=st)
                sd = work.tile([128, 1], F32, tag="lnsd")
                nc.scalar.activation(
                    out=sd, in_=mv[:, 1:2], func=mybir.ActivationFunctionType.Sqrt,
                    bias=eps_t, scale=1.0,
                )
                r = work.tile([128, 1], F32, tag="lnr")
                nc.vector.reciprocal(out=r, in_=sd)
                z = work.tile([128, A], F32, tag="lnz")
                nc.vector.tensor_scalar(
                    out=z, in0=ps, scalar1=mv[:, 0:1], scalar2=r,
                    op0=mybir.AluOpType.subtract, op1=mybir.AluOpType.mult,
                )
                zg = work.tile([128, A], F32, tag="lnzg")
                nc.vector.tensor_tensor(out=zg, in0=z, in1=gB, op=mybir.AluOpType.mult)
                nc.vector.tensor_tensor(out=dst, in0=zg, in1=bB, op=mybir.AluOpType.add)

            # ---------- adapt_in = LN(x @ pre_w.T), then AllGather #1 ----------
            pre_banks = [
                lambda: ps_out.tile([128, A], F32, tag="fout", name="pre_ps0"),
                lambda: ps_out.tile([128, A], F32, tag="fout", name="pre_ps1"),
                lambda: ps_sm.tile([128, A], F32, tag="sm", name="pre_ps2"),
                lambda: ps_acc.tile([128, A], F32, tag="adps", name="pre_ps3"),
            ]
            for sc in range(SC):
                ps = pre_banks[sc]()
                for k in range(KD):
                    nc.tensor.matmul(
                        ps, xT[:, k, sc * 128:(sc + 1) * 128], pre_w[:, k, :],
                        start=(k == 0), stop=(k == KD - 1),
                    )
                layernorm_to(ps, AI_tok[:, sc, :])
            # AG1 trigger (gpsimd queue; staging + transposes are interleaved
            # into the up/gate loop on the scalar queue — gpsimd is blocked on
            # the AG0 entry barrier until ~75us and vector can't issue DMAs)
            nc.scalar.dma_start(
                out=ag1_in[:].rearrange("(sc p) a -> p sc a", p=128), in_=AI_tok
            )
            nc.gpsimd.collective_compute(
                "AllGather", mybir.AluOpType.bypass, replica_groups=RG,
                ins=[ag1_in[:]], outs=[ag1_out[:]],
            )
            AIfull = acts.tile([128, TC_FULL, A], BF16)   # gathered AI token-major
            nc.gpsimd.dma_start(
                out=AIfull, in_=ag1_out[:].rearrange("(k p) a -> p k a", p=128)
            )

            # ---------- expert path, part 1 (injected into the up/gate loop) ---
            # per-expert h = AI @ adapter_w.T: matmul into borrowed ps_out
            # banks, stats + bf16 evacuation immediately (all table-free), so
            # PSUM frees fast. The rsqrt/z-transform runs after AO-LN (part 2)
            # to share the Rsqrt activation table.
            hsb = acts.tile([128, SC, E * A], BF16)   # h, token-major, e-outer
            mv8s = work.tile([128, SC, E, 2], F32, tag="mv8s", bufs=1)

            def expert_mm(sc):
                hp0 = ps_out.tile([128, 512], F32, tag="fout")
                hp1 = ps_out.tile([128, 512], F32, tag="fout")
                sl = AIT[:, sc * 128:(sc + 1) * 128]
                nc.tensor.matmul(hp0, sl, adw[:, 0:512], start=True, stop=True)
                nc.tensor.matmul(hp1, sl, adw[:, 512:1024], start=True, stop=True)
                st8 = work.tile([128, E, 6], F32, tag="st8")
                hps = [hp0, hp0, hp0, hp0, hp1, hp1, hp1, hp1]
                for e in range(E):
                    nc.vector.bn_stats(
                        out=st8[:, e, :], in_=hps[e][:, (e % 4) * A:(e % 4 + 1) * A]
                    )
                for e in range(E):
                    nc.vector.bn_aggr(out=mv8s[:, sc, e, :], in_=st8[:, e, :])
                nc.scalar.copy(out=hsb[:, sc, 0:512], in_=hp0)
                nc.scalar.copy(out=hsb[:, sc, 512:1024], in_=hp1)

            def expert_ln(sc, r8all):
                rw8 = work.tile([128, E], F32, tag="rw8")
                nc.vector.tensor_tensor(
                    out=rw8, in0=r8all[:, sc, :], in1=ew_sb[:, sc, :],
                    op=mybir.AluOpType.mult,
                )
                nmrw = work.tile([128, E], F32, tag="nmrw")
                nc.vector.tensor_tensor(
                    out=nmrw, in0=mv8s[:, sc, :, 0], in1=rw8,
                    op=mybir.AluOpType.mult,
                )
                nc.vector.tensor_scalar(
                    out=nmrw, in0=nmrw, scalar1=-1.0, scalar2=None,
                    op0=mybir.AluOpType.mult,
                )
                # z~_e = h_e * (r*ew)_e - m*(r*ew)_e, written e-outer [s, e, c]
                # (DVE, not scalar: keeps the scalar queue free for the facc
                # evacuations and bmm silus that overlap this phase)
                zt = workbig.tile([128, E, A], BF16, tag="zt")
                for e in range(E):
                    nc.vector.tensor_scalar(
                        out=zt[:, e, :], in0=hsb[:, sc, e * A:(e + 1) * A],
                        scalar1=rw8[:, e:e + 1], scalar2=nmrw[:, e:e + 1],
                        op0=mybir.AluOpType.mult, op1=mybir.AluOpType.add,
                    )
                zg = workbig.tile([128, E, A], BF16, tag="ztg")
                nc.vector.tensor_tensor(
                    out=zg, in0=zt, in1=agB, op=mybir.AluOpType.mult
                )
                t1 = workbig.tile([128, 4, A], BF16, tag="sum1")
                nc.vector.tensor_tensor(
                    out=t1, in0=zg[:, 0:4, :], in1=zg[:, 4:8, :],
                    op=mybir.AluOpType.add,
                )
                t2 = work.tile([128, 2, A], BF16, tag="sum2")
                nc.vector.tensor_tensor(
                    out=t2, in0=t1[:, 0:2, :], in1=t1[:, 2:4, :],
                    op=mybir.AluOpType.add,
                )
                mx = work.tile([128, A], BF16, tag="mx")
                nc.vector.tensor_tensor(
                    out=mx, in0=t2[:, 0, :], in1=t2[:, 1, :], op=mybir.AluOpType.add
                )
                nc.vector.tensor_tensor(
                    out=mix_tok[:, sc, :], in0=mx, in1=bmix_sb[:, sc, :],
                    op=mybir.AluOpType.add,
                )

            # ---------- hidden = silu(x@gate.T) * (x@up.T), feature-major -------
            # post contraction (adapt_out pre-LN, feature-major) interleaved with
            # one-iteration delay so PE never waits on the DVE mul.
            po_ps = ps_po.tile([128, T], F32, tag="po")

            def post_step(k):
                nc.tensor.matmul(
                    po_ps, post_w[:, :, k], hid[:, k, :],
                    start=(k == 0), stop=(k == HC - 1),
                )

            for hc in range(HC):
                up_ps = ps_big.tile([128, T], F32, tag="mm")
                gt_ps = ps_big.tile([128, T], F32, tag="mm")
                for k in range(KD):
                    nc.tensor.matmul(
                        up_ps, ug_w[:, hc, k, 0:128], xT[:, k, :],
                        start=(k == 0), stop=(k == KD - 1),
                    )
                for k in range(KD):
                    nc.tensor.matmul(
                        gt_ps, ug_w[:, hc, k, 128:256], xT[:, k, :],
                        start=(k == 0), stop=(k == KD - 1),
                    )
                sg = work2.tile([128, T], BF16, tag="sg")
                nc.scalar.activation(
                    out=sg, in_=gt_ps, func=mybir.ActivationFunctionType.Silu
                )
                nc.vector.tensor_tensor(
                    out=hid[:, hc, :], in0=sg, in1=up_ps, op=mybir.AluOpType.mult
                )
                if hc > 0:
                    post_step(hc - 1)
                if hc <= 1:
                    # AI -> feature-major AIT (xbar transpose, scalar queue;
                    # needed by expert_mm from hc=3)
                    for sc in (2 * hc, 2 * hc + 1):
                        nc.scalar.dma_start_transpose(
                            out=AIT[:, sc * 128:(sc + 1) * 128],
                            in_=AI_tok[:, sc, :],
                        )
                if 3 <= hc <= 6:
                    expert_mm(hc - 3)
            post_step(HC - 1)

            # ---------- adapt_out LN, feature-major (stats via PE ones-matmul) --
            # The per-token rstd lives on a [1,T] row; DVE reciprocal there is
            # ~6ns/elem on a single lane (3.2us). Re-partition [1,T]->[128,4]
            # by DMA, take the reciprocal across 128 lanes, and DMA back.
            AOf = acts.tile([128, T], BF16)
            nc.scalar.copy(out=AOf, in_=po_ps)
            sqf = aoln.tile([128, T], BF16)
            nc.vector.tensor_tensor(out=sqf, in0=AOf, in1=AOf, op=mybir.AluOpType.mult)
            s0 = ps_sm.tile([1, T], F32, tag="sm")
            nc.tensor.matmul(s0, ones_col, AOf, start=True, stop=True)
            mean_b = aoln.tile([1, T], F32)
            nc.vector.tensor_scalar(
                out=mean_b, in0=s0, scalar1=1.0 / A, scalar2=None,
                op0=mybir.AluOpType.mult,
            )
            s1 = ps_acc.tile([1, T], F32, tag="adps")
            nc.tensor.matmul(s1, ones_col, sqf, start=True, stop=True)
            varf = aoln.tile([1, T], F32)
            nc.vector.tensor_scalar(
                out=varf, in0=s1, scalar1=1.0 / A, scalar2=None,
                op0=mybir.AluOpType.mult,
            )
            m2 = aoln.tile([1, T], F32, tag="fb")
            nc.vector.tensor_tensor(out=m2, in0=mean_b, in1=mean_b,
                                    op=mybir.AluOpType.mult)
            nc.vector.tensor_tensor(out=varf, in0=varf, in1=m2,
                                    op=mybir.AluOpType.subtract)
            sdf = aoln.tile([1, T], F32, tag="fa")
            nc.scalar.activation(
                out=sdf, in_=varf, func=mybir.ActivationFunctionType.Sqrt,
                bias=eps_t[0:1], scale=1.0,
            )
            # re-partition the [1,T] rows to [128,4], compute r and -m*r across
            # 128 lanes, bounce through DRAM, and broadcast back as [128,T]
            # via stride-0 DMA -- no PE broadcasts, no slow 1-lane reciprocal.
            sd_rs = aoln.tile([128, 4], F32)
            nc.scalar.dma_start(out=sd_rs, in_=sdf)
            m_rs = aoln.tile([128, 4], F32)
            nc.scalar.dma_start(out=m_rs, in_=mean_b)
            r_rs = aoln.tile([128, 4], F32)
            nc.vector.reciprocal(out=r_rs, in_=sd_rs)
            r_bf = aoln.tile([128, 4], BF16)
            nc.vector.tensor_copy(out=r_bf, in_=r_rs)
            nmr_rs = aoln.tile([128, 4], F32)
            nc.vector.tensor_tensor(out=nmr_rs, in0=m_rs, in1=r_rs,
                                    op=mybir.AluOpType.mult)
            nmr_bf = aoln.tile([128, 4], BF16)
            nc.vector.tensor_scalar(
                out=nmr_bf, in0=nmr_rs, scalar1=-1.0, scalar2=None,
                op0=mybir.AluOpType.mult,
            )
            nc.scalar.dma_start(out=rrow_d[:], in_=r_bf)
            nc.scalar.dma_start(out=nmrrow_d[:], in_=nmr_bf)
            rB = aoln.tile([128, T], BF16)
            nc.scalar.dma_start(
                out=rB, in_=bass.AP(tensor=rrow_d, offset=0, ap=[[0, 128], [1, T]])
            )
            nmrB = aoln.tile([128, T], BF16)
            nc.scalar.dma_start(
                out=nmrB,
                in_=bass.AP(tensor=nmrrow_d, offset=0, ap=[[0, 128], [1, T]]),
            )

            # final_down keeps the PE busy while the LN tail completes
            def final_down(dc):
                op = ps_out.tile([128, T], F32, tag="fout")
                for k in range(HC):
                    nc.tensor.matmul(
                        op, wfin[:, dc, k, :], hid[:, k, :],
                        start=(k == 0), stop=(k == HC - 1),
                    )
                nc.scalar.copy(out=facc[:, dc, :], in_=op)

            final_down(0)
            z1 = aoln.tile([128, T], BF16)
            nc.vector.tensor_tensor(out=z1, in0=AOf, in1=rB,
                                    op=mybir.AluOpType.mult)
            z2 = aoln.tile([128, T], BF16)
            nc.vector.tensor_tensor(out=z2, in0=z1, in1=nmrB,
                                    op=mybir.AluOpType.add)
            nc.vector.tensor_scalar(
                out=AOT, in0=z2, scalar1=ancol[:, 0:1], scalar2=ancol[:, 1:2],
                op0=mybir.AluOpType.mult, op1=mybir.AluOpType.add,
            )
            # stage + trigger AG2 (staging on scalar; AOTfull copies on sync;
            # mixedT transposes ride the gpsimd queue, which unblocks exactly
            # when the AG2 collective completes)
            nc.scalar.dma_start(out=ag2_in[:], in_=AOT)
            nc.gpsimd.collective_compute(
                "AllGather", mybir.AluOpType.bypass, replica_groups=RG,
                ins=[ag2_in[:]], outs=[ag2_out[:]],
            )

            # ---------- expert path, part 2 (shares the Sqrt table) ----------
            sd8all = work.tile([128, SC, E], F32, tag="sd8all", bufs=1)
            nc.scalar.activation(
                out=sd8all, in_=mv8s[:, :, :, 1],
                func=mybir.ActivationFunctionType.Sqrt, bias=eps_t, scale=1.0,
            )
            r8all = work.tile([128, SC, E], F32, tag="r8all", bufs=1)
            nc.vector.reciprocal(out=r8all, in_=sd8all)
            for sc in range(SC):
                expert_ln(sc, r8all)
            # mixed -> feature-major (sync queue: runs in the AG2 wait window)
            for sc in range(SC):
                nc.sync.dma_start_transpose(
                    out=mixedT[:, sc * 128:(sc + 1) * 128], in_=mix_tok[:, sc, :]
                )
            for c in range(GROUP):
                nc.gpsimd.dma_start(
                    out=AOTfull[:, c, :], in_=ag2_out[c * A:(c + 1) * A, :]
                )
            AOTf = AOTfull.rearrange("a c t -> a (c t)")

            for dc in range(1, DC):
                final_down(dc)

            # ---------- w = silu(clip(AI_loc @ AO_full.T)); adapt = w.T chain ---
            ad_ps = ps_acc.tile([128, T], F32, tag="adps")
            wts_buf = {}

            def bmm1_step(j):
                w_ps = ps_big.tile([128, T], F32, tag="mm")
                nc.tensor.matmul(
                    w_ps, AOTf[:, j * 128:(j + 1) * 128], AIT, start=True, stop=True
                )
                wc = work2.tile([128, T], BF16, tag="wc")
                nc.vector.tensor_scalar(
                    out=wc, in0=w_ps, scalar1=-5.0, scalar2=5.0,
                    op0=mybir.AluOpType.max, op1=mybir.AluOpType.min,
                )
                wt = wtp.tile([128, T], BF16, tag="wts")
                nc.scalar.activation(
                    out=wt, in_=wc, func=mybir.ActivationFunctionType.Silu
                )
                wts_buf[j] = wt

            # depth-2 software pipeline: bmm2(j) trails bmm1(j+2) so the PE
            # never waits on the clip/silu stages
            for j in range(TC_FULL):
                bmm1_step(j)
                if j >= 2:
                    nc.tensor.matmul(
                        ad_ps, AIfull[:, j - 2, :], wts_buf.pop(j - 2),
                        start=(j - 2 == 0), stop=False,
                    )
            for j in (TC_FULL - 2, TC_FULL - 1):
                nc.tensor.matmul(
                    ad_ps, AIfull[:, j, :], wts_buf.pop(j),
                    start=False, stop=(j == TC_FULL - 1),
                )
            nc.scalar.copy(out=adaptT, in_=ad_ps)

            # ---------- finish output ----------
            def final_close(dc):
                op = ps_big.tile([128, T], F32, tag="mm")
                nc.tensor.matmul(
                    op, wfin[:, dc, HC, :], adaptT,
                    start=True, stop=False,
                )
                nc.tensor.matmul(
                    op, wfin[:, dc, HC + 1, :], mixedT,
                    start=False, stop=True,
                )
                ob = evac.tile([128, T], F32, tag="ob")
                nc.vector.tensor_tensor(
                    out=ob, in0=facc[:, dc, :], in1=op, op=mybir.AluOpType.add
                )
                nc.sync.dma_start(out=out_d[dc * 128:(dc + 1) * 128, :], in_=ob)

            for dc in range(DC):
                final_close(dc)

    nc.compile()
    return nc


def kernel(
    x, expert_weights, up_w, gate_w, down_w, pre_w, post_w, an_g, an_b,
    adapt_proj_w, adapter_w, adapter_g, adapter_b, expert_proj_w, output_proj_w,
):
    x = np.asarray(x, np.float32)
    expert_weights = np.asarray(expert_weights, np.float32)
    bf = ml_dtypes.bfloat16

    if "nc" not in _CACHE:
        _CACHE["nc"] = _build()
    nc = _CACHE["nc"]

    def pack(w, kc):
        # [kc*128, F] -> [128, kc, F] (partition-major SBUF layout)
        f = w.shape[1]
        return np.ascontiguousarray(
            w.reshape(kc, 128, f).transpose(1, 0, 2)
        ).astype(bf)

    ug_wT = np.concatenate(
        [np.asarray(up_w, np.float32), np.asarray(gate_w, np.float32)], axis=0
    ).T                                                        # [D, 2H]
    # hc-major pack: [128, HC, KD*256]; per hc: KD chunks of (up 128 | gate 128)
    up_part = ug_wT[:, :H].reshape(KD, 128, HC, 128)
    gt_part = ug_wT[:, H:].reshape(KD, 128, HC, 128)
    ug_hc = np.ascontiguousarray(
        np.stack([up_part, gt_part], axis=3)        # [k, p, hc, 2, 128]
        .transpose(1, 2, 0, 3, 4)                   # [p, hc, k, 2, 128]
        .reshape(128, HC, KD * 256)
    ).astype(bf)
    pre_wT = np.asarray(pre_w, np.float32).T                   # [D, A]
    post_pack = np.ascontiguousarray(
        np.asarray(post_w, np.float32).T.reshape(HC, 128, A).transpose(1, 2, 0)
    ).astype(bf)                                               # [128, A, HC]
    adapter_wT = (
        np.asarray(adapter_w, np.float32).transpose(2, 0, 1).reshape(A, E * A)
    ).astype(bf)                                               # [A, E*A] (e-major)
    down_w = np.asarray(down_w, np.float32)
    w_da = 0.1 * (down_w @ np.asarray(adapt_proj_w, np.float32))       # [D, A]
    w_mo = np.asarray(output_proj_w, np.float32) @ np.asarray(
        expert_proj_w, np.float32
    )                                                                   # [D, A]
    wfin = np.concatenate([down_w.T, w_da.T, w_mo.T], axis=0)  # [2304, D]
    angb = np.stack(
        [np.asarray(an_g, np.float32), np.asarray(an_b, np.float32)], axis=0
    )                                                                   # [2, A]
    ancol = np.ascontiguousarray(angb.T)                                # [A, 2]
    ag_row = np.asarray(adapter_g, np.float32).reshape(1, A * E).astype(bf)  # e-major
    bias_mix = (expert_weights @ np.asarray(adapter_b, np.float32)).astype(bf)

    xf = x.reshape(N, D)
    shared = {
        "ug_hc": ug_hc, "pre_wT": pack(pre_wT, KD),
        "post_wT": post_pack, "adapter_wT": adapter_wT,
        "wfin": np.ascontiguousarray(
            wfin.reshape(HC + 2, 128, DC, 128).transpose(1, 2, 0, 3)
            .reshape(128, DC, (HC + 2) * 128)
        ).astype(bf), "angb": angb, "ancol": ancol,
        "ag_row": ag_row,
    }
    in_maps = []
    for c in range(NCORES):
        sl = slice(c * T, (c + 1) * T)
        ewc = np.ascontiguousarray(expert_weights[sl]).reshape(SC, 128, E)
        bmc = np.ascontiguousarray(bias_mix[sl]).reshape(SC, 128, A)
        in_maps.append(
            dict(
                shared,
                xT=pack(np.ascontiguousarray(xf[sl].T), KD),
                ew=np.ascontiguousarray(ewc.transpose(1, 0, 2)),
                bias_mix=np.ascontiguousarray(bmc.transpose(1, 0, 2)),
            )
        )

    try:
        res = run_bass_kernel_spmd(nc, in_maps, list(range(NCORES))).results
    except Exception:
        # axon workers occasionally hang up; one retry on a fresh dispatch
        import time

        time.sleep(10)
        res = run_bass_kernel_spmd(nc, in_maps, list(range(NCORES))).results
    out = np.empty((N, D), np.float32)
    for c in range(NCORES):
        out[c * T:(c + 1) * T] = res[c]["out"].T
    return out.reshape(B, S, D)

